# revision 1
# baseline (speedup 1.0000x reference)
"""Criss-cross attention (2-stream) Trainium2 kernel.

Data-parallel over batch B=8 across 8 NeuronCores; one image pair per core.

Per-core algorithm (all matmuls bf16, fp32 PSUM accumulation):
  - q/k projections for both streams in one pass (bias host-corrected)
  - transposed logits E^T per column (diag mask PRELOADED into PSUM via a
    matmul so the logit matmuls accumulate on top of it) / per row,
    joint softmax without max-subtraction (logits are O(30); exp safe in f32)
  - Z-trick: Z[c',p] = sum_g x[c',g] * Phat[p,g] using host-supplied
    spatially-transposed x copies, then one dense (gamma*wv) @ Z projection.
    v-bias folds out exactly because joint softmax weights sum to 1:
      attn = wv@Z + bv;  out = gamma*attn + x = (gamma*wv)@Z + (x + gamma*bv)
    with x~ = x + gamma*bv supplied by host (and bq' = bq - wq@(gamma*bv),
    bk' = bk - wk@(gamma*bv) correcting the q/k projections).

Perf notes vs the original version:
  - exp/ln share one activation-table set (see _patch_act_tables) - no
    table reloads.
  - diag mask applied by PE (PSUM preload) instead of DVE adds.
  - 2-bank PSUM tiles everywhere: one egress instruction per 768-1024
    elements instead of per 384-512.
  - stationary operands padded to 128 columns where contiguous (row
    logits, ones, mask) to engage fast-weight-load.
  - rrep computed per 1024-slice (no [96,S] tile, half the instructions).
  - DMAs: fewer, larger, spread across SP-HWDGE / Pool-SWDGE / ACT-HWDGE.
"""

import sys

sys.path.insert(0, "/opt/trn_rl_repo")

import numpy as np
import ml_dtypes


def _patch_act_tables():
    """Make Exp/Ln resolve to the combined natural_log_exp_and_others
    activation-table set so alternating ln/exp does not reload tables.

    bass picks the first set whose membership contains the function; by
    default Exp -> exp_and_others and Ln -> natural_log, which forces a
    ~2.7us table reload on every ln<->exp switch.  Dropping Exp/Ln from
    the other sets' advertised membership makes both resolve to the one
    real hardware set that contains both functions."""
    import functools
    import concourse.hw_specs as hw_specs
    from concourse import mybir

    if getattr(hw_specs.get_activation_tables, "_cc_patched", False):
        return
    orig = hw_specs.get_activation_tables.__wrapped__
    EXP = mybir.ActivationFunctionType.Exp
    LN = mybir.ActivationFunctionType.Ln

    def patched(module_arch):
        out = {}
        for name, fns in orig(module_arch).items():
            fns = set(fns)
            if name != "natural_log_exp_and_others":
                fns.discard(EXP)
                fns.discard(LN)
            out[name] = fns
        return out

    wrapper = functools.cache(patched)
    wrapper._cc_patched = True
    hw_specs.get_activation_tables = wrapper


_patch_act_tables()

BF = ml_dtypes.bfloat16
B, C, H, W = 8, 256, 96, 96
CQ = 32
S = H * W  # 9216
NEG = -1.0e30
SLW = 24  # spatial slices per xT slab

_CACHE = {}


def build_nc(reps=1):
    import concourse.tile as tile
    from concourse import bacc, mybir

    f32 = mybir.dt.float32
    bf16 = mybir.dt.bfloat16

    nc = bacc.Bacc("TRN2", target_bir_lowering=False, debug=False, num_devices=8)

    din = {}

    def dparam(name, shape, dt=bf16):
        din[name] = nc.dram_tensor(name, shape, dt, kind="ExternalInput").ap()

    dparam("xa0", [C, S])          # bf16(x0 + g*bv0), channel-major
    dparam("xa1", [C, S])
    dparam("xtc0", [H, W * C])     # xtc[h, w*256+c] = x0[c,h,w]  (raw x)
    dparam("xtr0", [W, H * C])     # xtr[w, h*256+c] = x0[c,h,w]
    dparam("xtc1", [H, W * C])
    dparam("xtr1", [W, H * C])
    dparam("wqk", [C, 128])        # cols: wq0T|wk0T|wq1T|wk1T
    dparam("wva", [C, 512])        # (gamma*wv0).T | (gamma*wv1).T
    dparam("qkb", [128, 1], f32)   # bq0'|bk0'|bq1'|bk1'
    dparam("maskw", [H, 128])      # -1e30 diag (cols 96:128 zero)
    dparam("idm", [H, 384])        # identity tiled 4x along cols
    dparam("ident", [128, 128])    # identity (psum residual preload)
    out = nc.dram_tensor("out", [2, C, S], bf16, kind="ExternalOutput").ap()

    with tile.TileContext(nc) as tc:
        if reps == 1:
            _emit(tc, nc, din, out, mybir)
        else:
            with tc.For_i(0, reps, 1):
                _emit(tc, nc, din, out, mybir)

    nc.compile()
    return nc


def _emit(tc, nc, din, out, mybir):
    from contextlib import ExitStack

    f32 = mybir.dt.float32
    bf16 = mybir.dt.bfloat16
    EXP = mybir.ActivationFunctionType.Exp
    LN = mybir.ActivationFunctionType.Ln
    CPY = mybir.ActivationFunctionType.Copy
    ADD = mybir.AluOpType.add
    MUL = mybir.AluOpType.mult

    SP = S + 128  # padded q/k width (row-logit FWL reads h*96..h*96+128)

    ctx = ExitStack()
    with ctx:
        const = ctx.enter_context(tc.tile_pool(name="const", bufs=1))
        persist = ctx.enter_context(tc.tile_pool(name="persist", bufs=1))
        # PSUM: eps 2x2banks + lps 2x2banks = all 8 banks
        eps = ctx.enter_context(tc.tile_pool(name="eps", bufs=2, space="PSUM"))
        lps = ctx.enter_context(tc.tile_pool(name="lps", bufs=2, space="PSUM"))

        # ---------------- constants (ACT-triggered HWDGE queue) ----------
        wqk_t = []
        for kc in range(2):
            t = const.tile([128, 128], bf16, tag=f"wqk{kc}", name=f"wqk{kc}")
            nc.scalar.dma_start(t[:], din["wqk"][kc * 128:(kc + 1) * 128, :])
            wqk_t.append(t)
        # wv weights for both streams, one [128, 512] tile per kc half
        wva_t = []
        for kc in range(2):
            t = const.tile([128, 512], bf16, tag=f"wva{kc}", name=f"wva{kc}")
            nc.scalar.dma_start(t[:], din["wva"][kc * 128:(kc + 1) * 128, :])
            wva_t.append(t)
        wv_t = [
            [
                [wva_t[kc][:, s * 256 + mc * 128:s * 256 + (mc + 1) * 128]
                 for mc in range(2)]
                for kc in range(2)
            ]
            for s in range(2)
        ]
        qkb_t = const.tile([128, 1], f32, tag="qkb")
        nc.scalar.dma_start(qkb_t[:], din["qkb"][:])
        maskw_t = const.tile([H, 128], bf16, tag="maskw")
        nc.scalar.dma_start(maskw_t[:], din["maskw"][:])
        idm_t = const.tile([H, 384], bf16, tag="idm")
        nc.scalar.dma_start(idm_t[:], din["idm"][:])
        ones_t = const.tile([H, 128], bf16, tag="ones")
        nc.vector.memset(ones_t[:], 1.0)
        ident_t = const.tile([128, 128], bf16, tag="ident")
        nc.scalar.dma_start(ident_t[:], din["ident"][:])

        # persistent state
        qk_t = persist.tile([128, S], bf16, tag="qk")
        qs = persist.tile([CQ, SP], bf16, tag="qs")
        ks = persist.tile([CQ, SP], bf16, tag="ks")
        pcol = persist.tile([H, S], bf16, tag="pcol")
        prow = persist.tile([W, S], bf16, tag="prow")
        z = [persist.tile([128, S], bf16, tag=f"z{kc}", name=f"z{kc}")
             for kc in range(2)]

        # ---------------- q/k projections (xa streamed in 3K slabs) -----
        # qk_t rows: q0(0:32) k0(32:64) q1(64:96) k1(96:128)
        with tc.tile_pool(name="xslp", bufs=2) as xslp:
            for n3 in range(3):
                xsl = [[None] * 2 for _ in range(2)]
                for s in range(2):
                    for kc in range(2):
                        t = xslp.tile([128, 3072], bf16, tag=f"x{s}s{kc}",
                                      name=f"x{s}s{kc}")
                        nc.sync.dma_start(
                            t[:],
                            din[f"xa{s}"][kc * 128:(kc + 1) * 128,
                                          n3 * 3072:(n3 + 1) * 3072],
                        )
                        xsl[s][kc] = t
                for j2 in range(3):
                    n2 = n3 * 3 + j2
                    p = eps.tile([128, 1024], f32, tag="eps", name="p")
                    for b in range(2):
                        jsl = slice(j2 * 1024 + b * 512, j2 * 1024 + (b + 1) * 512)
                        pb = p[:, b * 512:(b + 1) * 512]
                        for kc in range(2):
                            nc.tensor.matmul(
                                pb[0:64, :], wqk_t[kc][:, 0:64],
                                xsl[0][kc][:, jsl],
                                start=(kc == 0), stop=(kc == 1),
                            )
                        for kc in range(2):
                            nc.tensor.matmul(
                                pb[64:128, :], wqk_t[kc][:, 64:128],
                                xsl[1][kc][:, jsl],
                                start=(kc == 0), stop=(kc == 1),
                                tile_position=(0, 64), skip_group_check=True,
                            )
                    nc.vector.tensor_scalar_add(
                        qk_t[:, n2 * 1024:(n2 + 1) * 1024], p[:], qkb_t[:]
                    )

        # transient pools for the attend phases
        slab = ctx.enter_context(tc.tile_pool(name="slab", bufs=3))
        tsl = ctx.enter_context(tc.tile_pool(name="tsl", bufs=2))
        resl = ctx.enter_context(tc.tile_pool(name="resl", bufs=2))
        obuf = ctx.enter_context(tc.tile_pool(name="obuf", bufs=2))

        # ---------------- attends (phase closures, interleaved emission) --
        # pixel (h,w) lives at free index w*96+h in pcol/prow
        prow_hw = prow[:].rearrange("p (w h) -> p h w", h=H)
        qs_wh = qs[:, 0:S].rearrange("p (h w) -> p w h", w=W)
        ks_wh = ks[:, 0:S].rearrange("p (h w) -> p w h", w=W)
        z_wh = [zz[:].rearrange("p (h w) -> p w h", w=W) for zz in z]

        def extract(a):
            # PE requires matmul operands at equal base partitions: copy the
            # attend's q/k blocks to partition-0-based tiles (SBUF->SBUF DMA)
            # with a zeroed 128-col pad for the FWL-padded row-logit reads.
            qr = 64 if a == 0 else 0    # query rows (q1 / q0)
            kr = 32 if a == 0 else 96   # key rows (k0 / k1)
            nc.gpsimd.dma_start(qs[:, 0:S], qk_t[qr:qr + 32, :])
            nc.gpsimd.dma_start(ks[:, 0:S], qk_t[kr:kr + 32, :])
            nc.vector.memset(qs[:, S:SP], 0.0)
            nc.vector.memset(ks[:, S:SP], 0.0)

        def stats_chunk(a, n2, dve_recip=False):
            sl = slice(n2 * 1024, (n2 + 1) * 1024)
            lt = lps.tile([128, 1024], f32, tag="lps", name="lt")
            for b in range(2):
                s512 = slice(n2 * 1024 + b * 512, n2 * 1024 + (b + 1) * 512)
                bank = lt[:, b * 512:(b + 1) * 512]
                nc.tensor.matmul(
                    bank, ones_t[:], pcol[:, s512],
                    start=True, stop=False, skip_group_check=True,
                )
                nc.tensor.matmul(
                    bank, ones_t[:], prow[:, s512],
                    start=False, stop=True, skip_group_check=True,
                )
            rs = tsl.tile([96, 1024], bf16, tag="rsl")
            if dve_recip:
                rr = tsl.tile([96, 1024], f32, tag="tln", name="rr")
                nc.vector.reciprocal_approx_fast(out=rr[:], in_=lt[0:96, :])
                nc.vector.tensor_copy(rs[:], rr[:])
            else:
                tl = tsl.tile([96, 1024], f32, tag="tln")
                nc.scalar.activation(tl[:], lt[0:96, :], LN)
                nc.scalar.activation(rs[:], tl[:], EXP, scale=-1.0)
            nc.vector.tensor_tensor(pcol[:, sl], pcol[:, sl], rs[:], MUL)
            nc.vector.tensor_tensor(prow[:, sl], prow[:, sl], rs[:], MUL)

        def logits(a, fuse_stats=False):
            # row-branch logits + exp (FWL-padded stationary) - emitted
            # FIRST so the joint-softmax stats (which need every row chunk
            # but only a prefix of col chunks) can interleave with the
            # col branch below.
            for h0 in range(0, H, 8):
                et = eps.tile([128, 1024], f32, tag="eps", name="et")
                for j in range(8):
                    h = h0 + j
                    off = (j // 4) * 512 + (j % 4) * 96
                    nc.tensor.matmul(
                        et[:, off:off + 96],
                        ks[:, h * 96:h * 96 + 128],
                        qs[:, h * 96:(h + 1) * 96],
                        start=True, stop=True, skip_group_check=True,
                    )
                src = (
                    et[0:96, :]
                    .rearrange("p (b x) -> p b x", x=512)[:, :, 0:384]
                    .rearrange("p b (j g) -> p b j g", g=W)
                )
                dst = prow_hw[:, h0:h0 + 8, :].rearrange(
                    "p (b j) w -> p b j w", b=2
                )
                nc.scalar.activation(dst, src, EXP)
            # column-branch: mask preload + logits + exp, with stats chunks
            # emitted as soon as their position coverage completes.
            n2_done = 0
            for i, w0 in enumerate(range(0, W, 8)):
                et = eps.tile([128, 1024], f32, tag="eps", name="et")
                for bk in range(2):
                    nc.tensor.matmul(
                        et[:, bk * 512:bk * 512 + 384], maskw_t[:], idm_t[:],
                        start=True, stop=False, skip_group_check=True,
                    )
                for j in range(8):
                    w = w0 + j
                    off = (j // 4) * 512 + (j % 4) * 96
                    nc.tensor.matmul(
                        et[0:96, off:off + 96],
                        ks_wh[:, w, :],
                        qs_wh[:, w, :],
                        start=False, stop=(j % 4 == 3), skip_group_check=True,
                    )
                src = et[0:96, :].rearrange("p (b x) -> p b x", x=512)[:, :, 0:384]
                nc.scalar.activation(
                    pcol[:, w0 * 96:(w0 + 8) * 96], src, EXP
                )
                if fuse_stats:
                    cov = (w0 + 8) * 96
                    while n2_done < 9 and (n2_done + 1) * 1024 <= cov:
                        stats_chunk(a, n2_done)
                        n2_done += 1
            if fuse_stats:
                while n2_done < 9:
                    stats_chunk(a, n2_done)
                    n2_done += 1

        def stats(a, dve_recip=False):
            for n2 in range(9):
                stats_chunk(a, n2, dve_recip)

        def zcol(a):
            # Z column branch: per w, Z[c', h]; scatter w-strided into z
            xtc = din[f"xtc{a}"][:].rearrange("p (w c) -> p w c", c=C)
            for w0 in range(0, W, SLW):
                xs = slab.tile([H, SLW * 256], bf16, tag="xslab", name="xs")
                nc.gpsimd.dma_start(xs[:], xtc[:, w0:w0 + SLW, :])
                for kc in range(2):
                    for g0 in range(0, SLW, 8):
                        zp = eps.tile([128, 1024], f32, tag="eps", name="zp")
                        for j in range(8):
                            wl = g0 + j
                            off = (j // 4) * 512 + (j % 4) * 96
                            nc.tensor.matmul(
                                zp[:, off:off + 96],
                                xs[:, wl * 256 + kc * 128:wl * 256 + kc * 128 + 128],
                                pcol[:, (w0 + wl) * 96:(w0 + wl + 1) * 96],
                                start=True, stop=True, skip_group_check=True,
                            )
                        src = (
                            zp[:, :]
                            .rearrange("p (b x) -> p b x", x=512)[:, :, 0:384]
                            .rearrange("p b (j g) -> p b j g", g=H)
                        )
                        dst = z_wh[kc][:, w0 + g0:w0 + g0 + 8, :].rearrange(
                            "p (b j) h -> p b j h", b=2
                        )
                        nc.scalar.activation(dst, src, CPY)

        def zrow(a):
            # Z row branch: per h, Z[c', w]; accumulate into z
            xtr = din[f"xtr{a}"][:].rearrange("p (h c) -> p h c", c=C)
            for h0 in range(0, H, SLW):
                xs = slab.tile([W, SLW * 256], bf16, tag="xslab", name="xs")
                nc.sync.dma_start(xs[:], xtr[:, h0:h0 + SLW, :])
                for kc in range(2):
                    for g0 in range(0, SLW, 8):
                        zp = eps.tile([128, 1024], f32, tag="eps", name="zp")
                        for j in range(8):
                            hl = g0 + j
                            off = (j // 4) * 512 + (j % 4) * 96
                            nc.tensor.matmul(
                                zp[:, off:off + 96],
                                xs[:, hl * 256 + kc * 128:hl * 256 + kc * 128 + 128],
                                prow_hw[:, h0 + hl, :],
                                start=True, stop=True, skip_group_check=True,
                            )
                        zsl = z[kc][
                            :, (h0 + g0) * 96:(h0 + g0 + 8) * 96
                        ].rearrange("p (b x) -> p b x", x=384)
                        src = zp[:, :].rearrange(
                            "p (b x) -> p b x", x=512
                        )[:, :, 0:384]
                        nc.vector.tensor_tensor(zsl, src, zsl, ADD)

        def proj(a, act_split=False):
            # final projection + residual + store.  act_split routes mc=1's
            # residual add through PE (identity-matmul PSUM preload) + ACT
            # copy so the DVE-bound tail of the last attend is shared.
            for mc in range(2):
                act_path = act_split and mc == 1
                for n3 in range(3):
                    ob = obuf.tile([128, 3072], bf16, tag="ob")
                    rt = resl.tile([128, 3072], bf16, tag="res", name="rt")
                    nc.sync.dma_start(
                        rt[:],
                        din[f"xa{a}"][mc * 128:(mc + 1) * 128,
                                      n3 * 3072:(n3 + 1) * 3072],
                    )
                    for j2 in range(3):
                        n2 = n3 * 3 + j2
                        op = lps.tile([128, 1024], f32, tag="lps", name="op")
                        for b in range(2):
                            s512 = slice(n2 * 1024 + b * 512,
                                         n2 * 1024 + (b + 1) * 512)
                            opb = op[:, b * 512:(b + 1) * 512]
                            if act_path:
                                nc.tensor.matmul(
                                    opb, ident_t[:],
                                    rt[:, j2 * 1024 + b * 512:
                                       j2 * 1024 + (b + 1) * 512],
                                    start=True, stop=False,
                                    skip_group_check=True,
                                )
                            for kc in range(2):
                                nc.tensor.matmul(
                                    opb, wv_t[a][kc][mc], z[kc][:, s512],
                                    start=(not act_path and kc == 0),
                                    stop=(kc == 1),
                                )
                        jsl = slice(j2 * 1024, (j2 + 1) * 1024)
                        if act_path:
                            nc.scalar.activation(ob[:, jsl], op[:], CPY)
                        else:
                            nc.vector.tensor_tensor(
                                ob[:, jsl], op[:], rt[:, jsl], ADD
                            )
                    nc.gpsimd.dma_start(
                        out[a, mc * 128:(mc + 1) * 128,
                            n3 * 3072:(n3 + 1) * 3072],
                        ob[:],
                    )

        # Emission order: attend 0's DVE-heavy projection is emitted between
        # attend 1's ACT-heavy logits and stats so the engine queues overlap
        # (proj0's adds run on DVE while ACT drains attend 1's exps; the z
        # tiles are reused WAR-safely because zcol(1) is emitted after
        # proj(0), and pcol/prow reuse is WAR-ordered behind zcol/zrow(0)).
        extract(0)
        logits(0, fuse_stats=True)
        zcol(0)
        zrow(0)
        proj(0)
        extract(1)
        logits(1, fuse_stats=True)
        zcol(1)
        zrow(1)
        proj(1)


def prep_inputs(inputs):
    """Host-side per-core input prep (numpy)."""
    g = float(np.asarray(inputs["gamma"]).reshape(-1)[0])
    maskw = np.zeros((H, 128), np.float32)
    np.fill_diagonal(maskw[:, 0:H], NEG)
    idm = np.tile(np.eye(H, dtype=np.float32), (1, 4))
    ident = np.eye(128, dtype=np.float32)
    wqk = np.concatenate(
        [inputs["wq0"].T, inputs["wk0"].T, inputs["wq1"].T, inputs["wk1"].T],
        axis=1,
    ).astype(BF)
    wv0 = (g * np.asarray(inputs["wv0"], np.float64)).T.astype(BF)
    wv1 = (g * np.asarray(inputs["wv1"], np.float64)).T.astype(BF)
    gb0 = g * np.asarray(inputs["bv0"], np.float64)
    gb1 = g * np.asarray(inputs["bv1"], np.float64)
    qkb = np.concatenate(
        [
            inputs["bq0"] - inputs["wq0"].astype(np.float64) @ gb0,
            inputs["bk0"] - inputs["wk0"].astype(np.float64) @ gb0,
            inputs["bq1"] - inputs["wq1"].astype(np.float64) @ gb1,
            inputs["bk1"] - inputs["wk1"].astype(np.float64) @ gb1,
        ]
    ).astype(np.float32)[:, None]
    maps = []
    for b in range(B):
        x0 = np.asarray(inputs["x0"][b], np.float32)
        x1 = np.asarray(inputs["x1"][b], np.float32)
        maps.append({
            "xa0": (x0 + np.float32(gb0[:, None, None])).reshape(C, S).astype(BF),
            "xa1": (x1 + np.float32(gb1[:, None, None])).reshape(C, S).astype(BF),
            "xtc0": np.ascontiguousarray(x0.transpose(1, 2, 0)).reshape(H, W * C).astype(BF),
            "xtr0": np.ascontiguousarray(x0.transpose(2, 1, 0)).reshape(W, H * C).astype(BF),
            "xtc1": np.ascontiguousarray(x1.transpose(1, 2, 0)).reshape(H, W * C).astype(BF),
            "xtr1": np.ascontiguousarray(x1.transpose(2, 1, 0)).reshape(W, H * C).astype(BF),
            "wqk": wqk, "wva": np.concatenate([wv0, wv1], axis=1), "qkb": qkb,
            "maskw": maskw.astype(BF), "idm": idm.astype(BF),
            "ident": ident.astype(BF),
        })
    return maps


def postprocess(results):
    cat0 = np.empty((B, C, H, W), np.float32)
    cat1 = np.empty((B, C, H, W), np.float32)
    for b in range(B):
        o = np.asarray(results[b]["out"]).astype(np.float32).reshape(2, C, H, W)
        cat0[b] = o[0]
        cat1[b] = o[1]
    return (cat0, cat1)


def kernel(**inputs):
    from concourse.bass_utils import run_bass_kernel_spmd

    if "nc" not in _CACHE:
        _CACHE["nc"] = build_nc()
    nc = _CACHE["nc"]
    maps = prep_inputs(inputs)
    res = run_bass_kernel_spmd(nc, maps, core_ids=list(range(B)))
    return postprocess(res.results)



# revision 25
# speedup vs baseline: 1.7784x; 1.7784x over previous
"""Criss-cross attention (2-stream) Trainium2 kernel.

Data-parallel over batch B=8 across 8 NeuronCores; one image pair per core.

Per-core algorithm (all matmuls bf16, fp32 PSUM accumulation):
  - q/k projections for both streams in one pass (bias host-corrected)
  - transposed logits E^T per column (diag mask PRELOADED into PSUM via a
    matmul so the logit matmuls accumulate on top of it) / per row,
    joint softmax without max-subtraction (logits are O(30); exp safe in f32)
  - Z-trick: Z[c',p] = sum_g x[c',g] * Phat[p,g] using host-supplied
    spatially-transposed x copies, then one dense (gamma*wv) @ Z projection.
    v-bias folds out exactly because joint softmax weights sum to 1:
      attn = wv@Z + bv;  out = gamma*attn + x = (gamma*wv)@Z + (x + gamma*bv)
    with x~ = x + gamma*bv supplied by host (and bq' = bq - wq@(gamma*bv),
    bk' = bk - wk@(gamma*bv) correcting the q/k projections).

Perf notes vs the original version:
  - exp/ln share one activation-table set (see _patch_act_tables) - no
    table reloads.
  - diag mask applied by PE (PSUM preload) instead of DVE adds.
  - 2-bank PSUM tiles everywhere: one egress instruction per 768-1024
    elements instead of per 384-512.
  - stationary operands padded to 128 columns where contiguous (row
    logits, ones, mask) to engage fast-weight-load.
  - rrep computed per 1024-slice (no [96,S] tile, half the instructions).
  - DMAs: fewer, larger, spread across SP-HWDGE / Pool-SWDGE / ACT-HWDGE.
"""

import sys

sys.path.insert(0, "/opt/trn_rl_repo")

import numpy as np
import ml_dtypes


def _patch_act_tables():
    """Make Exp/Ln resolve to the combined natural_log_exp_and_others
    activation-table set so alternating ln/exp does not reload tables.

    bass picks the first set whose membership contains the function; by
    default Exp -> exp_and_others and Ln -> natural_log, which forces a
    ~2.7us table reload on every ln<->exp switch.  Dropping Exp/Ln from
    the other sets' advertised membership makes both resolve to the one
    real hardware set that contains both functions."""
    import functools
    import concourse.hw_specs as hw_specs
    from concourse import mybir

    if getattr(hw_specs.get_activation_tables, "_cc_patched", False):
        return
    orig = hw_specs.get_activation_tables.__wrapped__
    EXP = mybir.ActivationFunctionType.Exp
    LN = mybir.ActivationFunctionType.Ln

    def patched(module_arch):
        out = {}
        for name, fns in orig(module_arch).items():
            fns = set(fns)
            if name != "natural_log_exp_and_others":
                fns.discard(EXP)
                fns.discard(LN)
            out[name] = fns
        return out

    wrapper = functools.cache(patched)
    wrapper._cc_patched = True
    hw_specs.get_activation_tables = wrapper


_patch_act_tables()

BF = ml_dtypes.bfloat16
B, C, H, W = 8, 256, 96, 96
CQ = 32
S = H * W  # 9216
NEG = -1.0e30
SLW = 24  # spatial slices per xT slab

_CACHE = {}


def build_nc(reps=1):
    import concourse.tile as tile
    from concourse import bacc, mybir

    f32 = mybir.dt.float32
    bf16 = mybir.dt.bfloat16

    nc = bacc.Bacc("TRN2", target_bir_lowering=False, debug=False, num_devices=8)

    din = {}

    def dparam(name, shape, dt=bf16):
        din[name] = nc.dram_tensor(name, shape, dt, kind="ExternalInput").ap()

    dparam("xa0", [C, S])          # bf16(x0 + g*bv0), channel-major
    dparam("xa1", [C, S])
    dparam("xtc0", [H, W * C])     # xtc[h, w*256+c] = x0[c,h,w]  (raw x)
    dparam("xtr0", [W, H * C])     # xtr[w, h*256+c] = x0[c,h,w]
    dparam("xtc1", [H, W * C])
    dparam("xtr1", [W, H * C])
    dparam("wqk", [C, 128])        # cols: wq0T|wk0T|wq1T|wk1T
    dparam("wva", [C, 512])        # (gamma*wv0).T | (gamma*wv1).T
    dparam("qkb", [128, 1], f32)   # bq0'|bk0'|bq1'|bk1'
    dparam("maskw", [H, 128])      # -1e30 diag (cols 96:128 zero)
    dparam("idm", [H, 384])        # identity tiled 4x along cols
    dparam("ident", [128, 128])    # identity (psum residual preload)
    out = nc.dram_tensor("out", [2, C, S], bf16, kind="ExternalOutput").ap()

    with tile.TileContext(nc) as tc:
        if reps == 1:
            _emit(tc, nc, din, out, mybir)
        else:
            with tc.For_i(0, reps, 1):
                _emit(tc, nc, din, out, mybir)

    nc.compile()
    return nc


def _emit(tc, nc, din, out, mybir):
    from contextlib import ExitStack

    f32 = mybir.dt.float32
    bf16 = mybir.dt.bfloat16
    EXP = mybir.ActivationFunctionType.Exp
    LN = mybir.ActivationFunctionType.Ln
    CPY = mybir.ActivationFunctionType.Copy
    ADD = mybir.AluOpType.add
    MUL = mybir.AluOpType.mult

    SP = S + 128  # padded q/k width (row-logit FWL reads h*96..h*96+128)

    ctx = ExitStack()
    with ctx:
        const = ctx.enter_context(tc.tile_pool(name="const", bufs=1))
        persist = ctx.enter_context(tc.tile_pool(name="persist", bufs=1))
        # PSUM: eps 2x2banks + lps 2x2banks = all 8 banks
        eps = ctx.enter_context(tc.tile_pool(name="eps", bufs=2, space="PSUM"))
        lps = ctx.enter_context(tc.tile_pool(name="lps", bufs=2, space="PSUM"))

        # ---------------- constants (ACT-triggered HWDGE queue) ----------
        wqk_t = []
        for kc in range(2):
            t = const.tile([128, 128], bf16, tag=f"wqk{kc}", name=f"wqk{kc}")
            nc.scalar.dma_start(t[:], din["wqk"][kc * 128:(kc + 1) * 128, :])
            wqk_t.append(t)
        # wv weights for both streams, one [128, 512] tile per kc half
        wva_t = []
        for kc in range(2):
            t = const.tile([128, 512], bf16, tag=f"wva{kc}", name=f"wva{kc}")
            nc.scalar.dma_start(t[:], din["wva"][kc * 128:(kc + 1) * 128, :])
            wva_t.append(t)
        wv_t = [
            [
                [wva_t[kc][:, s * 256 + mc * 128:s * 256 + (mc + 1) * 128]
                 for mc in range(2)]
                for kc in range(2)
            ]
            for s in range(2)
        ]
        qkb_t = const.tile([128, 1], f32, tag="qkb")
        nc.scalar.dma_start(qkb_t[:], din["qkb"][:])
        maskw_t = const.tile([H, 128], bf16, tag="maskw")
        nc.scalar.dma_start(maskw_t[:], din["maskw"][:])
        idm_t = const.tile([H, 384], bf16, tag="idm")
        nc.scalar.dma_start(idm_t[:], din["idm"][:])
        ones_t = const.tile([H, 128], bf16, tag="ones")
        nc.vector.memset(ones_t[:], 1.0)
        ident_t = const.tile([128, 128], bf16, tag="ident")
        nc.scalar.dma_start(ident_t[:], din["ident"][:])

        # persistent state
        qk_t = persist.tile([128, S], bf16, tag="qk")
        qs = persist.tile([CQ, SP], bf16, tag="qs")
        ks = persist.tile([CQ, SP], bf16, tag="ks")
        pcol = persist.tile([H, S], bf16, tag="pcol")
        prow = persist.tile([W, S], bf16, tag="prow")
        z = [persist.tile([128, S], bf16, tag=f"z{kc}", name=f"z{kc}")
             for kc in range(2)]

        # zero the FWL pad columns once up-front (extract DMAs never touch
        # them)
        nc.vector.memset(qs[:, S:SP], 0.0)
        nc.vector.memset(ks[:, S:SP], 0.0)

        # ---------------- q/k projections (xa streamed in 3K slabs) -----
        # qk_t rows: q0(0:32) k0(32:64) q1(64:96) k1(96:128)
        # attend 0's q/k extract is chunked into this loop, and attend 0's
        # row-logit blocks whose FWL-padded reads fit inside the extracted
        # prefix are emitted right behind each chunk, so PE/ACT start attend
        # 0 while xa is still streaming in.  (Called at emission time below;
        # row_block is late-bound.)
        ROWB = {0: (0, 8, 16), 1: (24, 32, 40, 48), 2: (56, 64, 72, 80, 88)}

        def qk_proj():
            with tc.tile_pool(name="xslp", bufs=2) as xslp:
                for n3 in range(3):
                    xsl = [[None] * 2 for _ in range(2)]
                    for s in range(2):
                        for kc in range(2):
                            t = xslp.tile([128, 3072], bf16, tag=f"x{s}s{kc}",
                                          name=f"x{s}s{kc}")
                            nc.sync.dma_start(
                                t[:],
                                din[f"xa{s}"][kc * 128:(kc + 1) * 128,
                                              n3 * 3072:(n3 + 1) * 3072],
                            )
                            xsl[s][kc] = t
                    for j2 in range(3):
                        n2 = n3 * 3 + j2
                        p = eps.tile([128, 1024], f32, tag="eps", name="p")
                        for b in range(2):
                            jsl = slice(j2 * 1024 + b * 512,
                                        j2 * 1024 + (b + 1) * 512)
                            pb = p[:, b * 512:(b + 1) * 512]
                            for kc in range(2):
                                nc.tensor.matmul(
                                    pb[0:64, :], wqk_t[kc][:, 0:64],
                                    xsl[0][kc][:, jsl],
                                    start=(kc == 0), stop=(kc == 1),
                                )
                            for kc in range(2):
                                nc.tensor.matmul(
                                    pb[64:128, :], wqk_t[kc][:, 64:128],
                                    xsl[1][kc][:, jsl],
                                    start=(kc == 0), stop=(kc == 1),
                                    tile_position=(0, 64),
                                    skip_group_check=True,
                                )
                        nc.vector.tensor_scalar_add(
                            qk_t[:, n2 * 1024:(n2 + 1) * 1024], p[:], qkb_t[:]
                        )
                    n3sl = slice(n3 * 3072, (n3 + 1) * 3072)
                    nc.scalar.dma_start(qs[:, n3sl], qk_t[64:96, n3sl])
                    nc.scalar.dma_start(ks[:, n3sl], qk_t[32:64, n3sl])
                    # row blocks ride the (idle-during-qk) lps pool so their
                    # ets — which wait on the extract DMA — never hold the
                    # eps slots the projection's p tiles cycle through.
                    for h0 in ROWB[n3]:
                        row_block(0, h0, lps, "lps")

        # transient pools for the attend phases are entered after qk_proj()
        # runs (xslp's address range frees for them); the closures below
        # late-bind these names.
        slab = tsl = resl = obuf = None

        # ---------------- attends (phase closures, interleaved emission) --
        # pixel (h,w) lives at free index w*96+h in pcol/prow
        prow_hw = prow[:].rearrange("p (w h) -> p h w", h=H)
        qs_wh = qs[:, 0:S].rearrange("p (h w) -> p w h", w=W)
        ks_wh = ks[:, 0:S].rearrange("p (h w) -> p w h", w=W)
        z_wh = [zz[:].rearrange("p (h w) -> p w h", w=W) for zz in z]

        def extract(a):
            # PE requires matmul operands at equal base partitions: copy the
            # attend's q/k blocks to partition-0-based tiles (SBUF->SBUF DMA
            # on the otherwise-idle ACT-triggered queue).
            # (attend 0's extract is chunked into the q/k projection loop.)
            qr = 64 if a == 0 else 0    # query rows (q1 / q0)
            kr = 32 if a == 0 else 96   # key rows (k0 / k1)
            nc.scalar.dma_start(qs[:, 0:S], qk_t[qr:qr + 32, :])
            nc.scalar.dma_start(ks[:, 0:S], qk_t[kr:kr + 32, :])

        def stats_chunk(a, n2, dve_recip=False):
            sl = slice(n2 * 1024, (n2 + 1) * 1024)
            lt = lps.tile([128, 1024], f32, tag="lps", name="lt")
            for b in range(2):
                s512 = slice(n2 * 1024 + b * 512, n2 * 1024 + (b + 1) * 512)
                bank = lt[:, b * 512:(b + 1) * 512]
                nc.tensor.matmul(
                    bank, ones_t[:], pcol[:, s512],
                    start=True, stop=False, skip_group_check=True,
                )
                nc.tensor.matmul(
                    bank, ones_t[:], prow[:, s512],
                    start=False, stop=True, skip_group_check=True,
                )
            rs = tsl.tile([96, 1024], bf16, tag="rsl")
            if dve_recip:
                rr = tsl.tile([96, 1024], f32, tag="tln", name="rr")
                nc.vector.reciprocal_approx_fast(out=rr[:], in_=lt[0:96, :])
                nc.vector.tensor_copy(rs[:], rr[:])
            else:
                tl = tsl.tile([96, 1024], f32, tag="tln")
                nc.scalar.activation(tl[:], lt[0:96, :], LN)
                nc.scalar.activation(rs[:], tl[:], EXP, scale=-1.0)
            nc.vector.tensor_tensor(pcol[:, sl], pcol[:, sl], rs[:], MUL)
            nc.vector.tensor_tensor(prow[:, sl], prow[:, sl], rs[:], MUL)

        def row_block(a, h0, pool=None, tag="eps"):
            # 8 row-branch logit matmuls (FWL-padded stationary) + exp
            et = (pool or eps).tile([128, 1024], f32, tag=tag, name="et")
            for j in range(8):
                h = h0 + j
                off = (j // 4) * 512 + (j % 4) * 96
                nc.tensor.matmul(
                    et[:, off:off + 96],
                    ks[:, h * 96:h * 96 + 128],
                    qs[:, h * 96:(h + 1) * 96],
                    start=True, stop=True, skip_group_check=True,
                )
            src = (
                et[0:96, :]
                .rearrange("p (b x) -> p b x", x=512)[:, :, 0:384]
                .rearrange("p b (j g) -> p b j g", g=W)
            )
            dst = prow_hw[:, h0:h0 + 8, :].rearrange(
                "p (b j) w -> p b j w", b=2
            )
            nc.scalar.activation(dst, src, EXP)

        def col_block(a, w0, pool, tag):
            # diag-mask PSUM preload + 8 col-branch logit matmuls + exp
            et = pool.tile([128, 1024], f32, tag=tag, name="et")
            for bk in range(2):
                nc.tensor.matmul(
                    et[:, bk * 512:bk * 512 + 384], maskw_t[:], idm_t[:],
                    start=True, stop=False, skip_group_check=True,
                )
            for j in range(8):
                w = w0 + j
                off = (j // 4) * 512 + (j % 4) * 96
                nc.tensor.matmul(
                    et[0:96, off:off + 96],
                    ks_wh[:, w, :],
                    qs_wh[:, w, :],
                    start=False, stop=(j % 4 == 3), skip_group_check=True,
                )
            src = et[0:96, :].rearrange("p (b x) -> p b x", x=512)[:, :, 0:384]
            nc.scalar.activation(
                pcol[:, w0 * 96:(w0 + 8) * 96], src, EXP
            )

        def zcol_slabs(a):
            # Z column branch: per w, Z[c', h]; scatter w-strided into z.
            # Generator yielding after each w-slab so attend 1's col logit
            # blocks can interleave (they chase this branch's pcol reads).
            xtc = din[f"xtc{a}"][:].rearrange("p (w c) -> p w c", c=C)
            for w0 in range(0, W, SLW):
                xs = slab.tile([H, SLW * 256], bf16, tag="xslab", name="xs")
                nc.gpsimd.dma_start(xs[:], xtc[:, w0:w0 + SLW, :])
                for kc in range(2):
                    for g0 in range(0, SLW, 8):
                        zp = eps.tile([128, 1024], f32, tag="eps", name="zp")
                        for j in range(8):
                            wl = g0 + j
                            off = (j // 4) * 512 + (j % 4) * 96
                            nc.tensor.matmul(
                                zp[:, off:off + 96],
                                xs[:, wl * 256 + kc * 128:wl * 256 + kc * 128 + 128],
                                pcol[:, (w0 + wl) * 96:(w0 + wl + 1) * 96],
                                start=True, stop=True, skip_group_check=True,
                            )
                        src = (
                            zp[:, :]
                            .rearrange("p (b x) -> p b x", x=512)[:, :, 0:384]
                            .rearrange("p b (j g) -> p b j g", g=H)
                        )
                        dst = z_wh[kc][:, w0 + g0:w0 + g0 + 8, :].rearrange(
                            "p (b j) h -> p b j h", b=2
                        )
                        nc.scalar.activation(dst, src, CPY)
                yield

        def zrow_slabs(a):
            # Z row branch: per h, Z[c', w]; accumulate into z.  Generator
            # yielding after each h-slab so proj emission can interleave.
            xtr = din[f"xtr{a}"][:].rearrange("p (h c) -> p h c", c=C)
            for h0 in range(0, H, SLW):
                xs = slab.tile([W, SLW * 256], bf16, tag="xslab", name="xs")
                nc.sync.dma_start(xs[:], xtr[:, h0:h0 + SLW, :])
                for kc in range(2):
                    for g0 in range(0, SLW, 8):
                        zp = eps.tile([128, 1024], f32, tag="eps", name="zp")
                        for j in range(8):
                            hl = g0 + j
                            off = (j // 4) * 512 + (j % 4) * 96
                            nc.tensor.matmul(
                                zp[:, off:off + 96],
                                xs[:, hl * 256 + kc * 128:hl * 256 + kc * 128 + 128],
                                prow_hw[:, h0 + hl, :],
                                start=True, stop=True, skip_group_check=True,
                            )
                        zsl = z[kc][
                            :, (h0 + g0) * 96:(h0 + g0 + 8) * 96
                        ].rearrange("p (b x) -> p b x", x=384)
                        src = zp[:, :].rearrange(
                            "p (b x) -> p b x", x=512
                        )[:, :, 0:384]
                        nc.vector.tensor_tensor(zsl, src, zsl, ADD)
                yield

        def proj_groups(a, act_split=False):
            # final projection + residual + store, as a generator yielding
            # after each (n3, mc) output group so callers can interleave other
            # phases' emissions between groups.  n3 is the outer loop so the
            # zrow interleave can release spatially-complete z ranges early.
            for n3 in range(3):
                for mc in range(2):
                    act_path = act_split and mc == 1
                    ob = obuf.tile([128, 3072], bf16, tag="ob")
                    rt = resl.tile([128, 3072], bf16, tag="res", name="rt")
                    nc.sync.dma_start(
                        rt[:],
                        din[f"xa{a}"][mc * 128:(mc + 1) * 128,
                                      n3 * 3072:(n3 + 1) * 3072],
                    )
                    for j2 in range(3):
                        n2 = n3 * 3 + j2
                        op = lps.tile([128, 1024], f32, tag="lps", name="op")
                        for b in range(2):
                            s512 = slice(n2 * 1024 + b * 512,
                                         n2 * 1024 + (b + 1) * 512)
                            opb = op[:, b * 512:(b + 1) * 512]
                            if act_path:
                                nc.tensor.matmul(
                                    opb, ident_t[:],
                                    rt[:, j2 * 1024 + b * 512:
                                       j2 * 1024 + (b + 1) * 512],
                                    start=True, stop=False,
                                    skip_group_check=True,
                                )
                            for kc in range(2):
                                nc.tensor.matmul(
                                    opb, wv_t[a][kc][mc], z[kc][:, s512],
                                    start=(not act_path and kc == 0),
                                    stop=(kc == 1),
                                )
                        jsl = slice(j2 * 1024, (j2 + 1) * 1024)
                        if act_path:
                            nc.scalar.activation(ob[:, jsl], op[:], CPY)
                        else:
                            nc.vector.tensor_tensor(
                                ob[:, jsl], op[:], rt[:, jsl], ADD
                            )
                    nc.gpsimd.dma_start(
                        out[a, mc * 128:(mc + 1) * 128,
                            n3 * 3072:(n3 + 1) * 3072],
                        ob[:],
                    )
                    yield

        def zrow_proj(a, next_rows=False):
            # zrow slabs interleaved with (optionally) the next attend's row
            # logit blocks — whose prow writes chase this branch's per-slab
            # reads — and with proj groups: proj group (n3, mc) needs z rows
            # h < (n3+1)*32 finalized, i.e. zrow slabs through
            # h0 = ceil((n3+1)*32 / SLW).  The residual path for mc=1 rides
            # PE (identity preload) + ACT copy so the DVE queue only carries
            # the zrow adds + mc=0 residuals.
            pg = proj_groups(a, act_split=True)
            zr = zrow_slabs(a)
            for si in range(4):          # slabs h0 = 0, 24, 48, 72
                next(zr)
                if next_rows:
                    for h0 in (24 * si, 24 * si + 8, 24 * si + 16):
                        row_block(1 - a, h0)
                if si >= 1:
                    next(pg)             # (n3 = si-1, mc = 0)
                    next(pg)             # (n3 = si-1, mc = 1)
            for _ in pg:
                pass

        # Emission program.  The two attends are pipelined across engines:
        #  - attend 0's q/k extract and row-logit blocks are chunked into the
        #    projection loop above, so ACT starts exp'ing while xa streams in.
        #  - attend 0 col blocks + fused stats (recip on DVE, normalize on
        #    GpSimd) follow; then extract(1) (WAR on qs/ks clears when
        #    attend 0's logit matmuls retire).
        #  - zcol(0) slabs interleave with attend 1's col blocks: the col
        #    exps overwrite pcol w-block by w-block right behind zcol(0)'s
        #    w-ordered reads, and attend 1's logit matmuls fill PE while
        #    zcol(0)'s groups pace on ACT egress.  Attend 1's ets live in
        #    the lps pool so the two pipelines don't share PSUM slots.
        #  - zrow(0) slabs interleave with attend 1's row blocks (same
        #    chasing argument) and with proj(0) groups.
        #  - proj residuals for mc=1 ride PE (identity preload) + ACT copy;
        #    mc=0 stays on DVE.
        qk_proj()
        slab = ctx.enter_context(tc.tile_pool(name="slab", bufs=3))
        tsl = ctx.enter_context(tc.tile_pool(name="tsl", bufs=2))
        resl = ctx.enter_context(tc.tile_pool(name="resl", bufs=2))
        obuf = ctx.enter_context(tc.tile_pool(name="obuf", bufs=2))
        n2_done = 0
        for i, w0 in enumerate(range(0, W, 8)):
            col_block(0, w0, eps, "eps")
            cov = (w0 + 8) * 96
            while n2_done < 9 and (n2_done + 1) * 1024 <= cov:
                stats_chunk(0, n2_done, dve_recip=True)
                n2_done += 1
        while n2_done < 9:
            stats_chunk(0, n2_done, dve_recip=True)
            n2_done += 1
        extract(1)
        zc = zcol_slabs(0)
        for si in range(4):
            next(zc)
            for w0 in (24 * si, 24 * si + 8, 24 * si + 16):
                col_block(1, w0, lps, "lps")
        zrow_proj(0, next_rows=True)
        for n2 in range(9):
            stats_chunk(1, n2, dve_recip=True)
        zc = zcol_slabs(1)
        for si in range(4):
            next(zc)
        zrow_proj(1)


def prep_inputs(inputs):
    """Host-side per-core input prep (numpy)."""
    g = float(np.asarray(inputs["gamma"]).reshape(-1)[0])
    maskw = np.zeros((H, 128), np.float32)
    np.fill_diagonal(maskw[:, 0:H], NEG)
    idm = np.tile(np.eye(H, dtype=np.float32), (1, 4))
    ident = np.eye(128, dtype=np.float32)
    wqk = np.concatenate(
        [inputs["wq0"].T, inputs["wk0"].T, inputs["wq1"].T, inputs["wk1"].T],
        axis=1,
    ).astype(BF)
    wv0 = (g * np.asarray(inputs["wv0"], np.float64)).T.astype(BF)
    wv1 = (g * np.asarray(inputs["wv1"], np.float64)).T.astype(BF)
    gb0 = g * np.asarray(inputs["bv0"], np.float64)
    gb1 = g * np.asarray(inputs["bv1"], np.float64)
    qkb = np.concatenate(
        [
            inputs["bq0"] - inputs["wq0"].astype(np.float64) @ gb0,
            inputs["bk0"] - inputs["wk0"].astype(np.float64) @ gb0,
            inputs["bq1"] - inputs["wq1"].astype(np.float64) @ gb1,
            inputs["bk1"] - inputs["wk1"].astype(np.float64) @ gb1,
        ]
    ).astype(np.float32)[:, None]
    maps = []
    for b in range(B):
        x0 = np.asarray(inputs["x0"][b], np.float32)
        x1 = np.asarray(inputs["x1"][b], np.float32)
        maps.append({
            "xa0": (x0 + np.float32(gb0[:, None, None])).reshape(C, S).astype(BF),
            "xa1": (x1 + np.float32(gb1[:, None, None])).reshape(C, S).astype(BF),
            "xtc0": np.ascontiguousarray(x0.transpose(1, 2, 0)).reshape(H, W * C).astype(BF),
            "xtr0": np.ascontiguousarray(x0.transpose(2, 1, 0)).reshape(W, H * C).astype(BF),
            "xtc1": np.ascontiguousarray(x1.transpose(1, 2, 0)).reshape(H, W * C).astype(BF),
            "xtr1": np.ascontiguousarray(x1.transpose(2, 1, 0)).reshape(W, H * C).astype(BF),
            "wqk": wqk, "wva": np.concatenate([wv0, wv1], axis=1), "qkb": qkb,
            "maskw": maskw.astype(BF), "idm": idm.astype(BF),
            "ident": ident.astype(BF),
        })
    return maps


def postprocess(results):
    cat0 = np.empty((B, C, H, W), np.float32)
    cat1 = np.empty((B, C, H, W), np.float32)
    for b in range(B):
        o = np.asarray(results[b]["out"]).astype(np.float32).reshape(2, C, H, W)
        cat0[b] = o[0]
        cat1[b] = o[1]
    return (cat0, cat1)


def kernel(**inputs):
    from concourse.bass_utils import run_bass_kernel_spmd

    if "nc" not in _CACHE:
        _CACHE["nc"] = build_nc()
    nc = _CACHE["nc"]
    maps = prep_inputs(inputs)
    res = run_bass_kernel_spmd(nc, maps, core_ids=list(range(B)))
    return postprocess(res.results)



# revision 36
# speedup vs baseline: 2.4595x; 1.3830x over previous
"""Criss-cross attention (2-stream) Trainium2 kernel.

Data-parallel over batch B=8 across 8 NeuronCores; one image pair per core.

Per-core algorithm (all matmuls bf16, fp32 PSUM accumulation):
  - q/k projections for both streams in one pass (bias host-corrected)
  - transposed logits E^T per column (diag mask PRELOADED into PSUM via a
    matmul so the logit matmuls accumulate on top of it) / per row,
    joint softmax without max-subtraction (logits are O(30); exp safe in f32)
  - Z-trick: Z[c',p] = sum_g x[c',g] * Phat[p,g] using host-supplied
    spatially-transposed x copies, then one dense (gamma*wv) @ Z projection.
    v-bias folds out exactly because joint softmax weights sum to 1:
      attn = wv@Z + bv;  out = gamma*attn + x = (gamma*wv)@Z + (x + gamma*bv)
    with x~ = x + gamma*bv supplied by host (and bq' = bq - wq@(gamma*bv),
    bk' = bk - wk@(gamma*bv) correcting the q/k projections).

Perf notes vs the original version:
  - exp/ln share one activation-table set (see _patch_act_tables) - no
    table reloads.
  - diag mask applied by PE (PSUM preload) instead of DVE adds.
  - 2-bank PSUM tiles everywhere: one egress instruction per 768-1024
    elements instead of per 384-512.
  - stationary operands padded to 128 columns where contiguous (row
    logits, ones, mask) to engage fast-weight-load.
  - rrep computed per 1024-slice (no [96,S] tile, half the instructions).
  - DMAs: fewer, larger, spread across SP-HWDGE / Pool-SWDGE / ACT-HWDGE.
"""

import sys

sys.path.insert(0, "/opt/trn_rl_repo")

import numpy as np
import ml_dtypes


def _patch_act_tables():
    """Make Exp/Ln resolve to the combined natural_log_exp_and_others
    activation-table set so alternating ln/exp does not reload tables.

    bass picks the first set whose membership contains the function; by
    default Exp -> exp_and_others and Ln -> natural_log, which forces a
    ~2.7us table reload on every ln<->exp switch.  Dropping Exp/Ln from
    the other sets' advertised membership makes both resolve to the one
    real hardware set that contains both functions."""
    import functools
    import concourse.hw_specs as hw_specs
    from concourse import mybir

    if getattr(hw_specs.get_activation_tables, "_cc_patched", False):
        return
    orig = hw_specs.get_activation_tables.__wrapped__
    EXP = mybir.ActivationFunctionType.Exp
    LN = mybir.ActivationFunctionType.Ln
    IDENT = mybir.ActivationFunctionType.Identity

    def patched(module_arch):
        out = {}
        for name, fns in orig(module_arch).items():
            fns = set(fns)
            if name != "natural_log_exp_and_others":
                fns.discard(EXP)
                fns.discard(LN)
                fns.discard(IDENT)
            out[name] = fns
        return out

    wrapper = functools.cache(patched)
    wrapper._cc_patched = True
    hw_specs.get_activation_tables = wrapper


_patch_act_tables()

BF = ml_dtypes.bfloat16
B, C, H, W = 8, 256, 96, 96
CQ = 32
S = H * W  # 9216
NEG = -1.0e30
SLW = 24  # spatial slices per xT slab

_CACHE = {}


def build_nc(reps=1):
    import concourse.tile as tile
    from concourse import bacc, mybir

    f32 = mybir.dt.float32
    bf16 = mybir.dt.bfloat16

    nc = bacc.Bacc("TRN2", target_bir_lowering=False, debug=False, num_devices=8)

    din = {}

    def dparam(name, shape, dt=bf16):
        din[name] = nc.dram_tensor(name, shape, dt, kind="ExternalInput").ap()

    dparam("xa0", [C, S])          # bf16(x0 + g*bv0), channel-major
    dparam("xa1", [C, S])
    dparam("xtc0", [H, W * C])     # xtc[h, w*256+c] = x0[c,h,w]  (raw x)
    dparam("xtr0", [W, H * C])     # xtr[w, h*256+c] = x0[c,h,w]
    dparam("xtc1", [H, W * C])
    dparam("xtr1", [W, H * C])
    dparam("wqk", [C, 128])        # cols: wq0T|wk0T|wq1T|wk1T
    dparam("wva", [C, 512])        # (gamma*wv0).T | (gamma*wv1).T
    dparam("qkb", [128, 1], f32)   # bq0'|bk0'|bq1'|bk1'
    dparam("maskw", [H, 128])      # -1e30 diag (cols 96:128 zero)
    dparam("idm", [H, 384])        # identity tiled 4x along cols
    dparam("ident", [128, 128])    # identity (psum residual preload)
    out = nc.dram_tensor("out", [2, C, S], bf16, kind="ExternalOutput").ap()

    with tile.TileContext(nc) as tc:
        if reps == 1:
            _emit(tc, nc, din, out, mybir)
        else:
            with tc.For_i(0, reps, 1, staggered_reset=True):
                _emit(tc, nc, din, out, mybir)

    nc.compile()
    return nc


def _emit(tc, nc, din, out, mybir):
    from contextlib import ExitStack

    f32 = mybir.dt.float32
    bf16 = mybir.dt.bfloat16
    EXP = mybir.ActivationFunctionType.Exp
    LN = mybir.ActivationFunctionType.Ln
    CPY = mybir.ActivationFunctionType.Copy
    IDENT = mybir.ActivationFunctionType.Identity
    ADD = mybir.AluOpType.add
    MUL = mybir.AluOpType.mult

    SP = S + 128  # padded q/k width (row-logit FWL reads h*96..h*96+128)

    ctx = ExitStack()
    with ctx:
        const = ctx.enter_context(tc.tile_pool(name="const", bufs=1))
        persist = ctx.enter_context(tc.tile_pool(name="persist", bufs=1))
        # PSUM: eps 2x2banks + lps 2x2banks = all 8 banks
        eps = ctx.enter_context(tc.tile_pool(name="eps", bufs=2, space="PSUM"))
        lps = ctx.enter_context(tc.tile_pool(name="lps", bufs=2, space="PSUM"))

        # ---------------- constants (ACT-triggered HWDGE queue) ----------
        wqk_t = []
        for kc in range(2):
            t = const.tile([128, 128], bf16, tag=f"wqk{kc}", name=f"wqk{kc}")
            nc.scalar.dma_start(t[:], din["wqk"][kc * 128:(kc + 1) * 128, :])
            wqk_t.append(t)
        # wv weights for both streams, one [128, 512] tile per kc half
        wva_t = []
        for kc in range(2):
            t = const.tile([128, 512], bf16, tag=f"wva{kc}", name=f"wva{kc}")
            nc.scalar.dma_start(t[:], din["wva"][kc * 128:(kc + 1) * 128, :])
            wva_t.append(t)
        wv_t = [
            [
                [wva_t[kc][:, s * 256 + mc * 128:s * 256 + (mc + 1) * 128]
                 for mc in range(2)]
                for kc in range(2)
            ]
            for s in range(2)
        ]
        qkb_t = const.tile([128, 1], f32, tag="qkb")
        nc.scalar.dma_start(qkb_t[:], din["qkb"][:])
        maskw_t = const.tile([H, 128], bf16, tag="maskw")
        nc.scalar.dma_start(maskw_t[:], din["maskw"][:])
        idm_t = const.tile([H, 384], bf16, tag="idm")
        nc.scalar.dma_start(idm_t[:], din["idm"][:])
        ones_t = const.tile([H, 128], bf16, tag="ones")
        nc.vector.memset(ones_t[:], 1.0)
        ident_t = const.tile([128, 128], bf16, tag="ident")
        nc.scalar.dma_start(ident_t[:], din["ident"][:])

        # persistent state
        qk_t = persist.tile([128, S], bf16, tag="qk")
        qs = persist.tile([CQ, SP], bf16, tag="qs")
        ks = persist.tile([CQ, SP], bf16, tag="ks")
        pcol = persist.tile([H, S], bf16, tag="pcol")
        prow = persist.tile([W, S], bf16, tag="prow")
        z = [persist.tile([128, S], bf16, tag=f"z{kc}", name=f"z{kc}")
             for kc in range(2)]

        # zero the FWL pad columns once up-front (extract DMAs never touch
        # them)
        nc.vector.memset(qs[:, S:SP], 0.0)
        nc.vector.memset(ks[:, S:SP], 0.0)

        # ---------------- q/k projections (xa streamed in 3K slabs) -----
        # qk_t rows: q0(0:32) k0(32:64) q1(64:96) k1(96:128)
        # attend 0's q/k extract is chunked into this loop, and attend 0's
        # row-logit blocks whose FWL-padded reads fit inside the extracted
        # prefix are emitted right behind each chunk, so PE/ACT start attend
        # 0 while xa is still streaming in.  (Called at emission time below;
        # row_block is late-bound.)
        ROWB = {0: (0, 8, 16), 1: (24, 32, 40, 48), 2: (56, 64, 72, 80, 88)}

        def qk_proj():
            with tc.tile_pool(name="xslp", bufs=2) as xslp:
                for n3 in range(3):
                    xsl = [[None] * 2 for _ in range(2)]
                    for s in range(2):
                        for kc in range(2):
                            t = xslp.tile([128, 3072], bf16, tag=f"x{s}s{kc}",
                                          name=f"x{s}s{kc}")
                            xsl[s][kc] = t
                            # alternate trigger queues: the qk phase is
                            # DMA-bound and gpsimd's queue is empty here
                            q = nc.sync if kc == 0 else nc.gpsimd
                            q.dma_start(
                                t[:],
                                din[f"xa{s}"][kc * 128:(kc + 1) * 128,
                                              n3 * 3072:(n3 + 1) * 3072],
                            )
                    for j2 in range(3):
                        n2 = n3 * 3 + j2
                        p = eps.tile([128, 1024], f32, tag="eps", name="p")
                        for b in range(2):
                            jsl = slice(j2 * 1024 + b * 512,
                                        j2 * 1024 + (b + 1) * 512)
                            pb = p[:, b * 512:(b + 1) * 512]
                            for kc in range(2):
                                nc.tensor.matmul(
                                    pb[0:64, :], wqk_t[kc][:, 0:64],
                                    xsl[0][kc][:, jsl],
                                    start=(kc == 0), stop=(kc == 1),
                                )
                            for kc in range(2):
                                nc.tensor.matmul(
                                    pb[64:128, :], wqk_t[kc][:, 64:128],
                                    xsl[1][kc][:, jsl],
                                    start=(kc == 0), stop=(kc == 1),
                                    tile_position=(0, 64),
                                    skip_group_check=True,
                                )
                        # PSUM egress + bias on DVE: ACT's queue holds the
                        # extract-gated row exps, so ACT egress here would
                        # starve the eps slots behind them (head-of-line).
                        nc.vector.tensor_scalar_add(
                            qk_t[:, n2 * 1024:(n2 + 1) * 1024], p[:], qkb_t[:]
                        )
                    n3sl = slice(n3 * 3072, (n3 + 1) * 3072)
                    nc.scalar.dma_start(qs[:, n3sl], qk_t[64:96, n3sl])
                    nc.scalar.dma_start(ks[:, n3sl], qk_t[32:64, n3sl])
                    # row blocks ride the (idle-during-qk) lps pool so their
                    # ets — which wait on the extract DMA — never hold the
                    # eps slots the projection's p tiles cycle through.
                    for h0 in ROWB[n3]:
                        row_block(0, h0, lps, "lps")

        # transient pools for the attend phases are entered after qk_proj()
        # runs (xslp's address range frees for them); the closures below
        # late-bind these names.
        slab = tsl = resl = obuf = None

        # ---------------- attends (phase closures, interleaved emission) --
        # pixel (h,w) lives at free index w*96+h in pcol/prow
        prow_hw = prow[:].rearrange("p (w h) -> p h w", h=H)
        qs_wh = qs[:, 0:S].rearrange("p (h w) -> p w h", w=W)
        ks_wh = ks[:, 0:S].rearrange("p (h w) -> p w h", w=W)
        z_wh = [zz[:].rearrange("p (h w) -> p w h", w=W) for zz in z]

        def extract(a):
            # PE requires matmul operands at equal base partitions: copy the
            # attend's q/k blocks to partition-0-based tiles (SBUF->SBUF DMA
            # on the otherwise-idle ACT-triggered queue).
            # (attend 0's extract is chunked into the q/k projection loop.)
            qr = 64 if a == 0 else 0    # query rows (q1 / q0)
            kr = 32 if a == 0 else 96   # key rows (k0 / k1)
            nc.scalar.dma_start(qs[:, 0:S], qk_t[qr:qr + 32, :])
            nc.scalar.dma_start(ks[:, 0:S], qk_t[kr:kr + 32, :])

        def stats_chunk(a, n2, dve_recip=False):
            sl = slice(n2 * 1024, (n2 + 1) * 1024)
            lt = lps.tile([128, 1024], f32, tag="lps", name="lt")
            for b in range(2):
                s512 = slice(n2 * 1024 + b * 512, n2 * 1024 + (b + 1) * 512)
                bank = lt[:, b * 512:(b + 1) * 512]
                nc.tensor.matmul(
                    bank, ones_t[:], pcol[:, s512],
                    start=True, stop=False, skip_group_check=True,
                )
                nc.tensor.matmul(
                    bank, ones_t[:], prow[:, s512],
                    start=False, stop=True, skip_group_check=True,
                )
            rs = tsl.tile([96, 1024], bf16, tag="rsl")
            if dve_recip:
                rr = tsl.tile([96, 1024], f32, tag="tln", name="rr")
                nc.vector.reciprocal_approx_fast(out=rr[:], in_=lt[0:96, :])
                nc.vector.tensor_copy(rs[:], rr[:])
            else:
                tl = tsl.tile([96, 1024], f32, tag="tln")
                nc.scalar.activation(tl[:], lt[0:96, :], LN)
                nc.scalar.activation(rs[:], tl[:], EXP, scale=-1.0)
            nc.vector.tensor_tensor(pcol[:, sl], pcol[:, sl], rs[:], MUL)
            nc.vector.tensor_tensor(prow[:, sl], prow[:, sl], rs[:], MUL)

        def row_block(a, h0, pool=None, tag="eps"):
            # 8 row-branch logit matmuls (FWL-padded stationary) + exp
            et = (pool or eps).tile([128, 1024], f32, tag=tag, name="et")
            for j in range(8):
                h = h0 + j
                off = (j // 4) * 512 + (j % 4) * 96
                nc.tensor.matmul(
                    et[:, off:off + 96],
                    ks[:, h * 96:h * 96 + 128],
                    qs[:, h * 96:(h + 1) * 96],
                    start=True, stop=True, skip_group_check=True,
                )
            src = (
                et[0:96, :]
                .rearrange("p (b x) -> p b x", x=512)[:, :, 0:384]
                .rearrange("p b (j g) -> p b j g", g=W)
            )
            dst = prow_hw[:, h0:h0 + 8, :].rearrange(
                "p (b j) w -> p b j w", b=2
            )
            nc.scalar.activation(dst, src, EXP)

        def col_block(a, w0, pool, tag):
            # diag-mask PSUM preload + 8 col-branch logit matmuls + exp
            et = pool.tile([128, 1024], f32, tag=tag, name="et")
            for bk in range(2):
                nc.tensor.matmul(
                    et[:, bk * 512:bk * 512 + 384], maskw_t[:], idm_t[:],
                    start=True, stop=False, skip_group_check=True,
                )
            for j in range(8):
                w = w0 + j
                off = (j // 4) * 512 + (j % 4) * 96
                nc.tensor.matmul(
                    et[0:96, off:off + 96],
                    ks_wh[:, w, :],
                    qs_wh[:, w, :],
                    start=False, stop=(j % 4 == 3), skip_group_check=True,
                )
            src = et[0:96, :].rearrange("p (b x) -> p b x", x=512)[:, :, 0:384]
            nc.scalar.activation(
                pcol[:, w0 * 96:(w0 + 8) * 96], src, EXP
            )

        def zcol_slabs(a):
            # Z column branch: per w, Z[c', h]; scatter w-strided into z.
            # Generator yielding after each w-slab so attend 1's col logit
            # blocks can interleave (they chase this branch's pcol reads).
            xtc = din[f"xtc{a}"][:].rearrange("p (w c) -> p w c", c=C)
            for w0 in range(0, W, SLW):
                xs = slab.tile([H, SLW * 256], bf16, tag="xslab", name="xs")
                nc.gpsimd.dma_start(xs[:], xtc[:, w0:w0 + SLW, :])
                for kc in range(2):
                    for g0 in range(0, SLW, 8):
                        zp = eps.tile([128, 1024], f32, tag="eps", name="zp")
                        for j in range(8):
                            wl = g0 + j
                            off = (j // 4) * 512 + (j % 4) * 96
                            nc.tensor.matmul(
                                zp[:, off:off + 96],
                                xs[:, wl * 256 + kc * 128:wl * 256 + kc * 128 + 128],
                                pcol[:, (w0 + wl) * 96:(w0 + wl + 1) * 96],
                                start=True, stop=True, skip_group_check=True,
                            )
                        src = (
                            zp[:, :]
                            .rearrange("p (b x) -> p b x", x=512)[:, :, 0:384]
                            .rearrange("p b (j g) -> p b j g", g=H)
                        )
                        dst = z_wh[kc][:, w0 + g0:w0 + g0 + 8, :].rearrange(
                            "p (b j) h -> p b j h", b=2
                        )
                        nc.scalar.activation(dst, src, CPY)
                yield

        def zrow_slabs(a):
            # Z row branch: per h, Z[c', w]; accumulate into z.  Generator
            # yielding after each h-slab so proj emission can interleave.
            xtr = din[f"xtr{a}"][:].rearrange("p (h c) -> p h c", c=C)
            for h0 in range(0, H, SLW):
                xs = slab.tile([W, SLW * 256], bf16, tag="xslab", name="xs")
                nc.sync.dma_start(xs[:], xtr[:, h0:h0 + SLW, :])
                for kc in range(2):
                    for g0 in range(0, SLW, 8):
                        zp = eps.tile([128, 1024], f32, tag="eps", name="zp")
                        for j in range(8):
                            hl = g0 + j
                            off = (j // 4) * 512 + (j % 4) * 96
                            nc.tensor.matmul(
                                zp[:, off:off + 96],
                                xs[:, hl * 256 + kc * 128:hl * 256 + kc * 128 + 128],
                                prow_hw[:, h0 + hl, :],
                                start=True, stop=True, skip_group_check=True,
                            )
                        zsl = z[kc][
                            :, (h0 + g0) * 96:(h0 + g0 + 8) * 96
                        ].rearrange("p (b x) -> p b x", x=384)
                        src = zp[:, :].rearrange(
                            "p (b x) -> p b x", x=512
                        )[:, :, 0:384]
                        nc.vector.tensor_tensor(zsl, src, zsl, ADD)
                yield

        def proj_groups(a, act_split=False):
            # final projection + residual + store, as a generator yielding
            # after each (n3, mc) output group so callers can interleave other
            # phases' emissions between groups.  n3 is the outer loop so the
            # zrow interleave can release spatially-complete z ranges early.
            for n3 in range(3):
                for mc in range(2):
                    act_path = act_split and mc == 1
                    ob = obuf.tile([128, 3072], bf16, tag="ob")
                    rt = resl.tile([128, 3072], bf16, tag="res", name="rt")
                    nc.sync.dma_start(
                        rt[:],
                        din[f"xa{a}"][mc * 128:(mc + 1) * 128,
                                      n3 * 3072:(n3 + 1) * 3072],
                    )
                    for j2 in range(3):
                        n2 = n3 * 3 + j2
                        op = lps.tile([128, 1024], f32, tag="lps", name="op")
                        for b in range(2):
                            s512 = slice(n2 * 1024 + b * 512,
                                         n2 * 1024 + (b + 1) * 512)
                            opb = op[:, b * 512:(b + 1) * 512]
                            if act_path:
                                nc.tensor.matmul(
                                    opb, ident_t[:],
                                    rt[:, j2 * 1024 + b * 512:
                                       j2 * 1024 + (b + 1) * 512],
                                    start=True, stop=False,
                                    skip_group_check=True,
                                )
                            for kc in range(2):
                                nc.tensor.matmul(
                                    opb, wv_t[a][kc][mc], z[kc][:, s512],
                                    start=(not act_path and kc == 0),
                                    stop=(kc == 1),
                                )
                        jsl = slice(j2 * 1024, (j2 + 1) * 1024)
                        if act_path:
                            nc.scalar.activation(ob[:, jsl], op[:], CPY)
                        else:
                            nc.vector.tensor_tensor(
                                ob[:, jsl], op[:], rt[:, jsl], ADD
                            )
                    nc.gpsimd.dma_start(
                        out[a, mc * 128:(mc + 1) * 128,
                            n3 * 3072:(n3 + 1) * 3072],
                        ob[:],
                    )
                    yield

        def zrow_proj(a, next_rows=False):
            # zrow slabs interleaved with (optionally) the next attend's row
            # logit blocks — whose prow writes chase this branch's per-slab
            # reads — and with proj groups: proj group (n3, mc) needs z rows
            # h < (n3+1)*32 finalized, i.e. zrow slabs through
            # h0 = ceil((n3+1)*32 / SLW).  The residual path for mc=1 rides
            # PE (identity preload) + ACT copy so the DVE queue only carries
            # the zrow adds + mc=0 residuals.
            pg = proj_groups(a, act_split=True)
            zr = zrow_slabs(a)
            for si in range(4):          # slabs h0 = 0, 24, 48, 72
                next(zr)
                if next_rows:
                    for h0 in (24 * si, 24 * si + 8, 24 * si + 16):
                        row_block(1 - a, h0)
                if si >= 1:
                    next(pg)             # (n3 = si-1, mc = 0)
                    next(pg)             # (n3 = si-1, mc = 1)
            for _ in pg:
                pass

        # Emission program.  The two attends are pipelined across engines:
        #  - attend 0's q/k extract and row-logit blocks are chunked into the
        #    projection loop above, so ACT starts exp'ing while xa streams in.
        #  - attend 0 col blocks + fused stats (recip on DVE, normalize on
        #    GpSimd) follow; then extract(1) (WAR on qs/ks clears when
        #    attend 0's logit matmuls retire).
        #  - zcol(0) slabs interleave with attend 1's col blocks: the col
        #    exps overwrite pcol w-block by w-block right behind zcol(0)'s
        #    w-ordered reads, and attend 1's logit matmuls fill PE while
        #    zcol(0)'s groups pace on ACT egress.  Attend 1's ets live in
        #    the lps pool so the two pipelines don't share PSUM slots.
        #  - zrow(0) slabs interleave with attend 1's row blocks (same
        #    chasing argument) and with proj(0) groups.
        #  - proj residuals for mc=1 ride PE (identity preload) + ACT copy;
        #    mc=0 stays on DVE.
        qk_proj()
        slab = ctx.enter_context(tc.tile_pool(name="slab", bufs=3))
        tsl = ctx.enter_context(tc.tile_pool(name="tsl", bufs=2))
        resl = ctx.enter_context(tc.tile_pool(name="resl", bufs=2))
        obuf = ctx.enter_context(tc.tile_pool(name="obuf", bufs=2))
        # stats0 uses the ACT ln/exp path (its window is DVE-bound with ACT
        # slack); stats1 uses the DVE reciprocal (its window is the reverse).
        n2_done = 0
        for i, w0 in enumerate(range(0, W, 8)):
            col_block(0, w0, eps, "eps")
            cov = (w0 + 8) * 96
            while n2_done < 9 and (n2_done + 1) * 1024 <= cov:
                stats_chunk(0, n2_done, dve_recip=True)
                n2_done += 1
        while n2_done < 9:
            stats_chunk(0, n2_done, dve_recip=True)
            n2_done += 1
        extract(1)
        zc = zcol_slabs(0)
        for si in range(4):
            next(zc)
            for w0 in (24 * si, 24 * si + 8, 24 * si + 16):
                col_block(1, w0, lps, "lps")
        zrow_proj(0, next_rows=True)
        for n2 in range(9):
            stats_chunk(1, n2, dve_recip=True)
        zc = zcol_slabs(1)
        for si in range(4):
            next(zc)
        zrow_proj(1)


def prep_inputs(inputs):
    """Host-side per-core input prep (numpy)."""
    g = float(np.asarray(inputs["gamma"]).reshape(-1)[0])
    maskw = np.zeros((H, 128), np.float32)
    np.fill_diagonal(maskw[:, 0:H], NEG)
    idm = np.tile(np.eye(H, dtype=np.float32), (1, 4))
    ident = np.eye(128, dtype=np.float32)
    wqk = np.concatenate(
        [inputs["wq0"].T, inputs["wk0"].T, inputs["wq1"].T, inputs["wk1"].T],
        axis=1,
    ).astype(BF)
    wv0 = (g * np.asarray(inputs["wv0"], np.float64)).T.astype(BF)
    wv1 = (g * np.asarray(inputs["wv1"], np.float64)).T.astype(BF)
    gb0 = g * np.asarray(inputs["bv0"], np.float64)
    gb1 = g * np.asarray(inputs["bv1"], np.float64)
    qkb = np.concatenate(
        [
            inputs["bq0"] - inputs["wq0"].astype(np.float64) @ gb0,
            inputs["bk0"] - inputs["wk0"].astype(np.float64) @ gb0,
            inputs["bq1"] - inputs["wq1"].astype(np.float64) @ gb1,
            inputs["bk1"] - inputs["wk1"].astype(np.float64) @ gb1,
        ]
    ).astype(np.float32)[:, None]
    maps = []
    for b in range(B):
        x0 = np.asarray(inputs["x0"][b], np.float32)
        x1 = np.asarray(inputs["x1"][b], np.float32)
        maps.append({
            "xa0": (x0 + np.float32(gb0[:, None, None])).reshape(C, S).astype(BF),
            "xa1": (x1 + np.float32(gb1[:, None, None])).reshape(C, S).astype(BF),
            "xtc0": np.ascontiguousarray(x0.transpose(1, 2, 0)).reshape(H, W * C).astype(BF),
            "xtr0": np.ascontiguousarray(x0.transpose(2, 1, 0)).reshape(W, H * C).astype(BF),
            "xtc1": np.ascontiguousarray(x1.transpose(1, 2, 0)).reshape(H, W * C).astype(BF),
            "xtr1": np.ascontiguousarray(x1.transpose(2, 1, 0)).reshape(W, H * C).astype(BF),
            "wqk": wqk, "wva": np.concatenate([wv0, wv1], axis=1), "qkb": qkb,
            "maskw": maskw.astype(BF), "idm": idm.astype(BF),
            "ident": ident.astype(BF),
        })
    return maps


def postprocess(results):
    cat0 = np.empty((B, C, H, W), np.float32)
    cat1 = np.empty((B, C, H, W), np.float32)
    for b in range(B):
        o = np.asarray(results[b]["out"]).astype(np.float32).reshape(2, C, H, W)
        cat0[b] = o[0]
        cat1[b] = o[1]
    return (cat0, cat1)


def kernel(**inputs):
    from concourse.bass_utils import run_bass_kernel_spmd

    if "nc" not in _CACHE:
        _CACHE["nc"] = build_nc()
    nc = _CACHE["nc"]
    maps = prep_inputs(inputs)
    res = run_bass_kernel_spmd(nc, maps, core_ids=list(range(B)))
    return postprocess(res.results)

